# revision 5
# baseline (speedup 1.0000x reference)
"""NT-Xent instance loss (nn_InstanceLoss) on 8 Trainium2 NeuronCores.

Symmetric ("compute each sim block once") sharding. z = concat(z_i, z_j)
has N=16384 rows, split into 16 kilobands of 1024 rows. Core c owns the
rows of kilobands c (band A) and c+8 (band B). Writing D = (col_kb -
row_kb) mod 16, core c computes sim blocks at distances D=0..8 for band
A and D=0..7 for band B: 17 kiloblocks of [1024, 1024] per core, i.e.
each off-diagonal block of the symmetric sim matrix is computed exactly
once somewhere, and every core does the same amount of work.

Each band is processed in [128 x 1024] PSUM tiles. For each tile of
exp(sim - C_band):
  - exp runs on the SCALAR engine (ACT, exact spline exp + free row-sum
    accumulator) for most m-tiles, and on the VECTOR engine for ~3/8
    via a saturating exp2 bit-trick: u16 = sat_round(sim*A+B) bitcast
    as bf16 gives e^(sim-C) to ~2-3% (mean-centered by DELTA; uint16
    saturation at 0 handles underflow). DVE tiles' row sums are bf16
    tensor_reduce ops, deferred by one tile.
  - column sums (contributions to the transposed block's rows, by
    symmetry) come from a ones-vector matmul on the tensor engine,
    accumulated across the band's 8 row-tiles into one PSUM bank
    holding 2 x 512 column-sum slots at partitions 0/32 (tile_position
    column-group trick). Colsum matmuls are emitted 3 m-slots BEHIND
    the sim matmuls so the in-order PE never waits for an exp output.
  - D=0 chunks produce no colsums (their transposed contributions are
    the row sums themselves): those matmuls/exports are skipped.

Diagonal masking: DVE-drained tiles of the D=0 chunks use
scalar_tensor_tensor with a [128,1024] mask tensor that lands -inf on
every column f with f%128==p (the true diag plus 7 harmless extra cells
per row, which the host adds back exactly). ACT-drained diag tiles get
a [128,128] -1e30 diag added on the DVE before the exp.

The host combines row sums, column sums (un-offset into absolute fp64)
and host-computed positives: lse = log(s), loss = mean(lse - pos), plus
the masked-cell corrections. A host-side exact fallback covers any row
that comes back non-finite.

Per-core uniformity trick: each core receives z rolled by -c*1024 rows
(pre-transposed / sqrt(2)-scaled / bf16-cast on the host), so the same
SPMD program works on every core. Positives are computed on the host in
fp64 (O(N*d)).

Schedule details vs the naive loop: the PE p-state is warmed with dummy
matmuls during the initial z DMA; the z head is DMA'd from the ACT
queue (ready ~3.5us before the sync queue); the last two chunks are
all-ACT so the DVE's deferred reduces don't serialize the tail.
"""

import math
import os as _os

import numpy as np
import ml_dtypes

TRAIN_NUM = 8192
EMBED = 128
N = 2 * TRAIN_NUM            # 16384
NCORES = 8
KB = 1024                    # kiloband height
M_PER_BAND = KB // 128       # 8 m-tiles per band
CHUNK = 1024                 # PSUM sim chunk (2 banks); 2 col-sum slots
NEG_BIG = -1.0e30
# E[max_j u.w_j] for ~16k gaussian w_j with per-coord var 2
ROWMAX_COEF = math.sqrt(2.0 * math.log(N - 1)) * math.sqrt(2.0)

# exp2 bit-trick constants for the DVE-offloaded exp tiles:
#   u16 = sat_round(x*A + (BF16_ONE + DELTA - C*A)); bitcast<bf16>(u16) ~ e^(x-C)
EXP2_A = 128.0 * math.log2(math.e)
BF16_ONE = 16256.0
EXP2_DELTA = -128.0 * math.log2(0.5 / (math.log(2.0) ** 2))

V2_STT = _os.environ.get("V2_STT", "1") == "1"
V2_ACTDMA = _os.environ.get("V2_ACTDMA", "1") == "1"
V2_WARMMM = _os.environ.get("V2_WARMMM", "1") == "1"
COLSUM_LAG = 3               # colsum matmuls trail sim matmuls by 3 m-slots

# chunk visit order per band: diag chunk (j=0) 4th so the start isn't
# DVE-heavy; band B last so the taper chunks end the kernel.
BAND_JS = [
    (0, 0, 0, [1, 2, 3, 0, 4, 5, 6, 7, 8]),   # band, row_base, col_base, js
    (1, 8192, 8192, [1, 2, 3, 0, 4, 5, 6, 7]),
]
# m-tiles drained on the DVE per (band, j); rest on ACT. Last two
# emitted chunks (band B j=6,7) are all-ACT so the tail has no DVE work.
DVE_M_DEFAULT = (2, 4, 6)
DVE_M_DIAG = (2, 4, 6)       # used by the host for mask corrections


def _dve_m(band, j):
    if band == 1 and j >= 6:
        return ()
    if j == 0:
        return DVE_M_DIAG
    return DVE_M_DEFAULT


# exported colsum chunks, in emission order (D=0 chunks export nothing)
EXPORTED = [(0, j) for j in [1, 2, 3, 4, 5, 6, 7, 8]] + [
    (1, j) for j in [1, 2, 3, 4, 5, 6, 7]
]
N_COLCHUNKS = len(EXPORTED)  # 15

_cached = None


def _build():
    import concourse.bacc as bacc
    import concourse.tile as tile
    from concourse import mybir

    nc = bacc.Bacc(
        "TRN2",
        target_bir_lowering=False,
        debug=False,
        num_devices=NCORES,
    )
    f32 = mybir.dt.float32
    bf16 = mybir.dt.bfloat16
    u16 = mybir.dt.uint16
    A = mybir.AluOpType

    zT_dram = nc.dram_tensor("zT", (EMBED, N), bf16, kind="ExternalInput")
    cneg_dram = nc.dram_tensor("c_neg", (128, 4), f32, kind="ExternalInput")
    dmask_dram = nc.dram_tensor("dmask", (128, 2048), f32, kind="ExternalInput")
    s_dram = nc.dram_tensor("s_out", (128, 16), f32, kind="ExternalOutput")
    col_dram = nc.dram_tensor(
        "col_out", (N_COLCHUNKS, 2, 512), f32, kind="ExternalOutput"
    )

    neg_np = np.zeros((128, 128), dtype=np.float32)
    np.fill_diagonal(neg_np, NEG_BIG)
    neg_dram = nc.inline_tensor(neg_np, name="neg_mask")
    ones_dram = nc.inline_tensor(
        np.ones((128, 1), dtype=ml_dtypes.bfloat16), name="ones_vec"
    )
    warm_dram = nc.inline_tensor(
        np.ones((128, 512), dtype=ml_dtypes.bfloat16), name="warm_src"
    )

    with tile.TileContext(nc) as tc:
        with (
            tc.tile_pool(name="zbuf", bufs=1) as zpool,
            tc.tile_pool(name="consts", bufs=1) as cpool,
            tc.tile_pool(name="persist", bufs=1) as perpool,
            tc.tile_pool(name="psum", bufs=3, space="PSUM") as ppool,
            tc.tile_pool(name="colsum", bufs=2, space="PSUM") as cspool,
            tc.tile_pool(name="expout", bufs=6) as epool,
            tc.tile_pool(name="expu16", bufs=6) as upool,
            tc.tile_pool(name="stage", bufs=3) as stpool,
        ):
            # small consts + the z head go out on the ACT hwdge queue,
            # which clears its preamble ~3.5us before the sync queue.
            _dmaq = nc.scalar if V2_ACTDMA else nc.sync
            warm_sb = cpool.tile([128, 512], bf16)
            _dmaq.dma_start(out=warm_sb, in_=warm_dram[:, :])
            ones_sb = cpool.tile([128, 1], bf16)
            _dmaq.dma_start(out=ones_sb, in_=ones_dram[:, :])
            cneg_sb = cpool.tile([128, 4], f32)
            _dmaq.dma_start(out=cneg_sb, in_=cneg_dram[:, :])

            z_sb = zpool.tile([EMBED, N], bf16)
            # head: lhsT rows 0..1024 + first chunk's cols 1024..2048
            _dmaq.dma_start(out=z_sb[:, 0:2048], in_=zT_dram[:, 0:2048])

            for qs, qe in [(2048, 4608), (4608, 9216), (9216, 12800), (12800, N)]:
                nc.sync.dma_start(out=z_sb[:, qs:qe], in_=zT_dram[:, qs:qe])
            negm = cpool.tile([128, 128], f32)
            nc.sync.dma_start(out=negm, in_=neg_dram[:, :])
            dmask = cpool.tile([128, 2048], f32)
            nc.sync.dma_start(out=dmask, in_=dmask_dram[:, :])

            # PE p-state warmup: ~6 x 512-col matmuls during the z DMA
            warmp = cspool.tile([128, 512], f32, tag="cs")
            for _ in range(6 if V2_WARMMM else 0):
                nc.tensor.matmul(
                    warmp[:, :],
                    lhsT=warm_sb[:, 0:128],
                    rhs=warm_sb[:, 0:512],
                    start=True,
                    stop=True,
                )

            # trigger the exp ACT-table load early so it overlaps the z DMA
            warm = cpool.tile([128, 1], f32)
            nc.scalar.activation(
                out=warm,
                in_=cneg_sb[:, 0:1],
                func=mybir.ActivationFunctionType.Exp,
                bias=cneg_sb[:, 0:1],
                scale=0.0,
            )

            # row-sum accumulator: [band*8+m, chunk] laid out as [128,16,9]
            s_band = perpool.tile([128, 16, 9], f32)
            nc.vector.memset(s_band, 0.0)
            s_out_sb = perpool.tile([128, 16], f32)

            # flat slot list: (band, j, m) in emission order
            slots = []
            for band, row_base, col_base, js in BAND_JS:
                for j in js:
                    for m in range(M_PER_BAND):
                        slots.append((band, row_base, col_base, j, m))

            cs_tiles = {}       # (band, j) -> colsum psum tile
            col_k = 0
            pending_reduce = []  # deferred DVE row-sum reduces (1-deep)
            exported_set = set(EXPORTED)

            def emit_colsum(qband, qj, qm, ev):
                nonlocal col_k
                if (qband, qj) not in exported_set:
                    return
                if qm == 0:
                    cs_t = cspool.tile([128, 512], f32, tag="cs")
                    cs_tiles[(qband, qj)] = cs_t
                colsum = cs_tiles[(qband, qj)]
                for s in range(2):
                    nc.tensor.matmul(
                        colsum[32 * s : 32 * s + 1, :],
                        lhsT=ones_sb,
                        rhs=ev[:, s * 512 : (s + 1) * 512],
                        start=(qm == 0),
                        stop=(qm == M_PER_BAND - 1),
                        tile_position=(0, 32 * s),
                    )
                if qm == M_PER_BAND - 1:
                    stage = stpool.tile([128, 512], f32)
                    nc.vector.tensor_copy(stage[0:33, :], colsum[0:33, :])
                    nc.sync.dma_start(
                        out=col_dram[col_k, :, :], in_=stage[0:33:32, :]
                    )
                    col_k += 1
                    del cs_tiles[(qband, qj)]

            ev_ring = {}        # slot index -> exp tile (for lagged colsums)
            band_last_slot = {0: 0, 1: 0}
            for i, (band, row_base, col_base, j, m) in enumerate(slots):
                band_last_slot[band] = i

            for i, (band, row_base, col_base, j, m) in enumerate(slots):
                r0 = row_base + m * 128
                cbase = col_base + j * CHUNK
                ps = ppool.tile([128, CHUNK], f32, tag="ps")
                for k in range(2):
                    nc.tensor.matmul(
                        ps[:, k * 512 : (k + 1) * 512],
                        lhsT=z_sb[:, r0 : r0 + 128],
                        rhs=z_sb[:, cbase + k * 512 : cbase + (k + 1) * 512],
                        start=True,
                        stop=True,
                    )
                # lagged colsum for the slot COLSUM_LAG back
                if i >= COLSUM_LAG:
                    qb, _, _, qj, qm = slots[i - COLSUM_LAG]
                    emit_colsum(qb, qj, qm, ev_ring.pop(i - COLSUM_LAG))

                slot_ap = s_band[:, band * 8 + m, j : j + 1]
                if m in _dve_m(band, j):
                    ut = upool.tile([128, CHUNK], u16, tag="expu")
                    if j == 0 and V2_STT:
                        # fused all-k diag mask (host corrects extras)
                        nc.vector.scalar_tensor_tensor(
                            out=ut,
                            in0=ps,
                            scalar=EXP2_A,
                            in1=dmask[:, band * 1024 : (band + 1) * 1024],
                            op0=A.mult,
                            op1=A.add,
                        )
                    else:
                        if j == 0:
                            nc.vector.tensor_add(
                                ps[:, m * 128 : m * 128 + 128],
                                ps[:, m * 128 : m * 128 + 128],
                                negm,
                            )
                        nc.vector.tensor_scalar(
                            out=ut,
                            in0=ps,
                            scalar1=EXP2_A,
                            scalar2=cneg_sb[:, 2 + band : 3 + band],
                            op0=A.mult,
                            op1=A.add,
                        )
                    ev = ut.bitcast(bf16)
                    # deferred row-sum reduce (1 tile deep) keeps the next
                    # PSUM drain at the head of the DVE queue
                    pending_reduce.append((ev, slot_ap))
                    if len(pending_reduce) > 1:
                        ev_d, slot_d = pending_reduce.pop(0)
                        nc.vector.tensor_reduce(
                            out=slot_d,
                            in_=ev_d,
                            axis=mybir.AxisListType.X,
                            op=A.add,
                        )
                else:
                    if j == 0:
                        # exact diag mask for ACT-drained diag tiles
                        nc.vector.tensor_add(
                            ps[:, m * 128 : m * 128 + 128],
                            ps[:, m * 128 : m * 128 + 128],
                            negm,
                        )
                    et = epool.tile([128, CHUNK], bf16, tag="exp")
                    nc.scalar.activation(
                        out=et,
                        in_=ps,
                        func=mybir.ActivationFunctionType.Exp,
                        bias=cneg_sb[:, band : band + 1],
                        scale=1.0,
                        accum_out=slot_ap,
                    )
                    ev = et
                ev_ring[i] = ev

                if i == band_last_slot[band]:
                    # flush this band's deferred reduces, fold + ship its
                    # row sums while the next band (or epilogue) runs
                    for ev_d, slot_d in pending_reduce:
                        nc.vector.tensor_reduce(
                            out=slot_d,
                            in_=ev_d,
                            axis=mybir.AxisListType.X,
                            op=A.add,
                        )
                    pending_reduce.clear()
                    b8 = band * 8
                    nc.vector.tensor_reduce(
                        out=s_out_sb[:, b8 : b8 + 8],
                        in_=s_band[:, b8 : b8 + 8, :],
                        axis=mybir.AxisListType.X,
                        op=A.add,
                    )
                    nc.sync.dma_start(
                        out=s_dram[:, b8 : b8 + 8], in_=s_out_sb[:, b8 : b8 + 8]
                    )

            # flush the lagged colsums of the final slots
            for q in range(len(slots) - COLSUM_LAG, len(slots)):
                qb, _, _, qj, qm = slots[q]
                emit_colsum(qb, qj, qm, ev_ring.pop(q))

    nc.compile()
    return nc


def _get_nc():
    global _cached
    if _cached is None:
        _cached = _build()
    return _cached


def _prep(z_i: np.ndarray, z_j: np.ndarray):
    z = np.concatenate(
        [np.asarray(z_i, np.float32), np.asarray(z_j, np.float32)], axis=0
    )
    w = z * np.float32(math.sqrt(2.0))  # fold 1/T=2 into both operands
    wnorm = np.linalg.norm(w.astype(np.float64), axis=1)
    # per-kiloband exp offset from extreme-value estimate of the row max
    c_band = np.array(
        [
            ROWMAX_COEF * np.median(wnorm[b * KB : (b + 1) * KB])
            for b in range(16)
        ],
        dtype=np.float64,
    )
    # all-k diag mask pattern: -big at every column f with f%128 == p
    dpat = np.zeros((128, 1024), dtype=np.float32)
    for k in range(8):
        dpat[np.arange(128), k * 128 + np.arange(128)] = NEG_BIG
    in_maps = []
    for c in range(NCORES):
        wc = np.roll(w, -c * KB, axis=0)
        zT = np.ascontiguousarray(wc.T).astype(ml_dtypes.bfloat16)
        cneg = np.zeros((128, 4), dtype=np.float32)
        cneg[:, 0] = -c_band[c]
        cneg[:, 1] = -c_band[c + 8]
        cneg[:, 2] = BF16_ONE + EXP2_DELTA - c_band[c] * EXP2_A
        cneg[:, 3] = BF16_ONE + EXP2_DELTA - c_band[c + 8] * EXP2_A
        dmask = np.concatenate(
            [dpat + cneg[0, 2], dpat + cneg[0, 3]], axis=1
        ).astype(np.float32)
        in_maps.append({"zT": zT, "c_neg": cneg, "dmask": dmask})
    return w, c_band, in_maps


def _finish(w, c_band, results):
    s_abs = np.zeros(N, dtype=np.float64)
    # positives on the host: O(N*d), negligible next to the device's O(N^2*d)
    w64 = w.astype(np.float64)
    pos = (w64 * np.roll(w64, -TRAIN_NUM, axis=0)).sum(axis=1)
    for c in range(NCORES):
        r = results[c]
        s_dev = r["s_out"].astype(np.float64)      # [128, 16]
        col_dev = r["col_out"].astype(np.float64)  # [15, 2, 512]
        for band, kb in ((0, c), (1, c + 8)):
            scale = math.exp(c_band[kb])
            # row sums: s_dev[p, band*8+m] -> band row m*128+p
            rows = s_dev[:, band * 8 : band * 8 + 8].T.reshape(KB)  # [8*128]
            g0 = kb * KB
            s_abs[g0 : g0 + KB] += rows * scale
        for k, (band, j) in enumerate(EXPORTED):
            kb = c if band == 0 else c + 8
            scale = math.exp(c_band[kb])
            for sl in range(2):
                L = band * 8192 + j * CHUNK + sl * 512
                vals = col_dev[k, sl, :] * scale
                g = (c * KB + L) % N
                s_abs[g : g + 512] += vals

    # add back the extra cells masked by the all-k diag trick: for each
    # kiloband's diag block, DVE-drained rows m in DVE_M_DIAG lost cells
    # (m*128+p, k*128+p) for k != m.
    for kb in (range(16) if V2_STT else ()):
        base = kb * KB
        for m in DVE_M_DIAG:
            R = base + m * 128 + np.arange(128)
            for k in range(8):
                if k == m:
                    continue
                C = base + k * 128 + np.arange(128)
                s_abs[R] += np.exp(
                    np.einsum("ij,ij->i", w64[R], w64[C])
                )

    with np.errstate(divide="ignore", invalid="ignore", over="ignore"):
        lse = np.log(s_abs)
    bad = ~np.isfinite(lse)
    if bad.any():
        idx = np.nonzero(bad)[0]
        wb = w[idx].astype(np.float64)
        sim_b = wb @ w64.T
        for ii, rr in enumerate(idx):
            sim_b[ii, rr] = -np.inf
        m_b = sim_b.max(axis=1)
        lse[idx] = np.log(np.exp(sim_b - m_b[:, None]).sum(axis=1)) + m_b
        pos_idx = np.where(idx < TRAIN_NUM, idx + TRAIN_NUM, idx - TRAIN_NUM)
        pos[idx] = np.einsum("ij,ij->i", wb, w64[pos_idx])
    loss = (lse - pos).mean()
    return np.float32(loss)


def run(z_i, z_j, trace=False, **kw):
    from concourse.bass_utils import run_bass_kernel_spmd

    nc = _get_nc()
    w, c_band, in_maps = _prep(z_i, z_j)
    res = run_bass_kernel_spmd(
        nc, in_maps, core_ids=list(range(NCORES)), trace=trace, **kw
    )
    return _finish(w, c_band, res.results), res


def kernel(z_i, z_j):
    loss, _ = run(z_i, z_j, trace=False)
    return loss


# revision 7
# speedup vs baseline: 1.2005x; 1.2005x over previous
"""NT-Xent instance loss (nn_InstanceLoss) on 8 Trainium2 NeuronCores.

Symmetric ("compute each sim block once") sharding. z = concat(z_i, z_j)
has N=16384 rows, split into 16 kilobands of 1024 rows. Core c owns the
rows of kilobands c (band A) and c+8 (band B). Writing D = (col_kb -
row_kb) mod 16, core c computes sim blocks at distances D=0..8 for band
A and D=0..7 for band B: 17 kiloblocks of [1024, 1024] per core, i.e.
each off-diagonal block of the symmetric sim matrix is computed exactly
once somewhere, and every core does the same amount of work.

Each band is processed in [128 x 1024] PSUM tiles (2 banks, 3-deep
pool so the PE can run ahead). For each tile of exp(sim - C_band):
  - exp runs on the SCALAR engine (ACT, exact spline exp + free row-sum
    accumulator) for ~2/3 of the m-tiles, and on the VECTOR engine for
    the rest via a saturating exp2 bit-trick: u16 = sat_round(sim*A+B)
    bitcast as bf16 gives e^(sim-C) to ~2-3% (mean-centered by DELTA;
    uint16 saturation at 0 handles underflow). DVE tiles' row sums are
    a deferred bf16 tensor_reduce. The split load-balances the two
    engines' measured rates (ACT ~1.4us/tile, DVE ~2.5us/tile).
  - column sums (contributions to the transposed block's rows, by
    symmetry) come from a ones-vector matmul on the tensor engine,
    accumulated across the band's 8 row-tiles into one PSUM bank
    holding 2 x 512 column-sum slots at partitions 0/32 (tile_position
    column-group trick).
The host combines row sums, column sums (un-offset into absolute fp64)
and host-computed positives: lse = log(s), loss = mean(lse - pos). D=0
column sums are discarded (already counted in the row sums).

exp offsets C_band are per-kiloband constants estimated on the host
from ||z_r|| (extreme-value statistics of gaussian dot products);
margins of ~±70 in the exponent make fp32/bf16 over/underflow
impossible for randn-like inputs, and a host-side exact fallback
covers any row that still comes back non-finite.

Per-core uniformity trick: each core receives z rolled by -c*1024 rows
(pre-transposed / sqrt(2)-scaled / bf16-cast on the host), so the same
SPMD program works on every core: band A = local rows 0..1024 vs local
cols 0..9216, band B = local rows 8192..9216 vs local cols 8192..16384,
self-diagonals at local col == local row. Positives (w_i . w_{i+n})
are O(N*d) and computed on the host in fp64.
"""

import math

import numpy as np
import ml_dtypes

TRAIN_NUM = 8192
EMBED = 128
N = 2 * TRAIN_NUM            # 16384
NCORES = 8
KB = 1024                    # kiloband height
M_PER_BAND = KB // 128       # 8 m-tiles per band
CHUNK = 1024                 # PSUM sim chunk (2 banks); 2 col-sum slots
NEG_BIG = -1.0e30
# E[max_j u.w_j] for ~16k gaussian w_j with per-coord var 2
ROWMAX_COEF = math.sqrt(2.0 * math.log(N - 1)) * math.sqrt(2.0)

# exp2 bit-trick constants for the DVE-offloaded exp tiles:
#   u16 = sat_round(x*A + (BF16_ONE + DELTA - C*A)); bitcast<bf16>(u16) ~ e^(x-C)
# A maps nats to bf16-exponent ulps; DELTA centers the linear-mantissa
# approximation so E[approx/exact] = 1 (the raw trick is biased by
# integral_0^1 (1+t)2^-t dt = 0.5/ln(2)^2 ~ 1.0407).
EXP2_A = 128.0 * math.log2(math.e)
BF16_ONE = 16256.0
EXP2_DELTA = -128.0 * math.log2(0.5 / (math.log(2.0) ** 2))
# m-tiles whose exp runs on the vector engine (rest on scalar/ACT), chosen
# per chunk to balance ACT ~1.4us/tile vs DVE ~2.5us/tile (measured), and
# spread through the chunk so ACT/DVE alternate on the three PSUM buffers
DVE_M_3 = (2, 5, 7)
DVE_M_4 = (1, 3, 5, 7)

# (band, chunk) list: band A covers local cols 0..9216 (9 chunks),
# band B covers local cols 8192..16384 (8 chunks); all uniform 1024 wide.
BANDS = [
    # (band_idx, row_base, col_base, widths)
    (0, 0, 0, [CHUNK] * 9),
    (1, 8192, 8192, [CHUNK] * 8),
]
N_COLCHUNKS = 15             # 8 + 7 col-sum tiles DMA'd out (D=0 skipped)
SLOTS_PER_BAND = 9           # s_band j-slots (band B leaves j=8 zero)

_cached = None


def _build():
    import concourse.bacc as bacc
    import concourse.tile as tile
    from concourse import mybir

    nc = bacc.Bacc(
        "TRN2",
        target_bir_lowering=False,
        debug=False,
        num_devices=NCORES,
    )
    f32 = mybir.dt.float32
    bf16 = mybir.dt.bfloat16

    u16 = mybir.dt.uint16
    zT_dram = nc.dram_tensor("zT", (EMBED, N), bf16, kind="ExternalInput")
    cneg_dram = nc.dram_tensor("c_neg", (128, 4), f32, kind="ExternalInput")
    s_dram = nc.dram_tensor("s_out", (128, 16), f32, kind="ExternalOutput")
    col_dram = nc.dram_tensor(
        "col_out", (N_COLCHUNKS, 2, 512), f32, kind="ExternalOutput"
    )

    neg_np = np.zeros((128, 128), dtype=np.float32)
    np.fill_diagonal(neg_np, NEG_BIG)
    neg_dram = nc.inline_tensor(neg_np, name="neg_mask")
    ones_dram = nc.inline_tensor(
        np.ones((128, 1), dtype=ml_dtypes.bfloat16), name="ones_vec"
    )

    with tile.TileContext(nc) as tc:
        with (
            tc.tile_pool(name="zbuf", bufs=1) as zpool,
            tc.tile_pool(name="consts", bufs=1) as cpool,
            tc.tile_pool(name="persist", bufs=1) as perpool,
            tc.tile_pool(name="psum", bufs=3, space="PSUM") as ppool,
            tc.tile_pool(name="colsum", bufs=2, space="PSUM") as cspool,
            tc.tile_pool(name="expout", bufs=6) as epool,
            tc.tile_pool(name="expu16", bufs=6) as upool,
            tc.tile_pool(name="stage", bufs=3) as stpool,
        ):
            # the z head gates the first matmul — issue it before everything;
            # consts follow (only needed once ACT/diag work starts)
            z_sb = zpool.tile([EMBED, N], bf16)
            nc.sync.dma_start(out=z_sb[:, 0:640], in_=zT_dram[:, 0:640])

            ones_sb = cpool.tile([128, 1], bf16)
            nc.sync.dma_start(out=ones_sb, in_=ones_dram[:, :])
            cneg_sb = cpool.tile([128, 4], f32)
            nc.sync.dma_start(out=cneg_sb, in_=cneg_dram[:, :])

            negm = cpool.tile([128, 128], f32)
            nc.sync.dma_start(out=negm, in_=neg_dram[:, :])

            # trigger the exp ACT-table load early so it overlaps the z DMA
            # instead of stalling the first real exp
            warm = cpool.tile([128, 1], f32)
            nc.scalar.activation(
                out=warm,
                in_=cneg_sb[:, 0:1],
                func=mybir.ActivationFunctionType.Exp,
                bias=cneg_sb[:, 0:1],
                scale=0.0,
            )

            cuts = [640, CHUNK, 4608, 9216, 12800, N]
            for qs, qe in zip(cuts, cuts[1:]):
                nc.sync.dma_start(
                    out=z_sb[:, qs:qe],
                    in_=zT_dram[:, qs:qe],
                )

            # row-sum accumulator: [band*8+m, chunk] laid out as [128,16,9];
            # band B writes only j=0..7, so zero the whole thing first
            s_band = perpool.tile([128, 16, SLOTS_PER_BAND], f32)
            nc.vector.memset(s_band, 0.0)
            s_out_sb = perpool.tile([128, 16], f32)

            col_k = 0
            for band, row_base, col_base, widths in BANDS:
                for j, width in enumerate(widths):
                    nslots = width // 512
                    cbase = col_base + j * CHUNK
                    colsum = None
                    if j != 0:
                        colsum = cspool.tile([128, 512], f32, tag="cs")
                    deferred = []
                    for m in range(M_PER_BAND):
                        r0 = row_base + m * 128
                        ps = ppool.tile([128, CHUNK], f32, tag="ps")
                        for k in range(nslots):
                            nc.tensor.matmul(
                                ps[:, k * 512 : (k + 1) * 512],
                                lhsT=z_sb[:, r0 : r0 + 128],
                                rhs=z_sb[:, cbase + k * 512 : cbase + (k + 1) * 512],
                                start=True,
                                stop=True,
                            )
                        if j == 0:
                            # self-similarity diag at chunk offset 128*m
                            nc.vector.tensor_add(
                                ps[:, m * 128 : m * 128 + 128],
                                ps[:, m * 128 : m * 128 + 128],
                                negm,
                            )
                        ci = band * 9 + j
                        if band == 1 and j >= 6:
                            dve_m = ()
                        elif ci % 3 == 1:
                            dve_m = DVE_M_4
                        else:
                            dve_m = DVE_M_3
                        if m in dve_m:
                            ut = upool.tile([128, CHUNK], u16, tag="expu")
                            nc.vector.tensor_scalar(
                                out=ut[:, :width],
                                in0=ps[:, :width],
                                scalar1=EXP2_A,
                                scalar2=cneg_sb[:, 2 + band : 3 + band],
                                op0=mybir.AluOpType.mult,
                                op1=mybir.AluOpType.add,
                            )
                            ev = ut.bitcast(bf16)
                            # row-sum reduces deferred to chunk end so the
                            # DVE FIFO doesn't delay the next tile's exp
                            deferred.append(
                                (ev, s_band[:, band * 8 + m, j : j + 1], width)
                            )
                        else:
                            et = epool.tile([128, CHUNK], bf16, tag="exp")
                            nc.scalar.activation(
                                out=et[:, :width],
                                in_=ps[:, :width],
                                func=mybir.ActivationFunctionType.Exp,
                                bias=cneg_sb[:, band : band + 1],
                                scale=1.0,
                                accum_out=s_band[:, band * 8 + m, j : j + 1],
                            )
                            ev = et
                        if j != 0:
                            for s in range(nslots):
                                nc.tensor.matmul(
                                    colsum[32 * s : 32 * s + 1, :],
                                    lhsT=ones_sb,
                                    rhs=ev[:, s * 512 : (s + 1) * 512],
                                    start=(m == 0),
                                    stop=(m == M_PER_BAND - 1),
                                    tile_position=(0, 32 * s),
                                )
                    if j != 0:
                        stage = stpool.tile([128, 512], f32)
                        nc.vector.tensor_copy(stage[0:33, :], colsum[0:33, :])
                        nc.sync.dma_start(
                            out=col_dram[col_k, :, :], in_=stage[0:33:32, :]
                        )
                        col_k += 1
                    # deferred row-sum reduces last: they may lag into the
                    # next chunk without holding PSUM or the colsum stage
                    for ev_d, slot_d, w_d in deferred:
                        nc.vector.tensor_reduce(
                            out=slot_d,
                            in_=ev_d[:, :w_d],
                            axis=mybir.AxisListType.X,
                            op=mybir.AluOpType.add,
                        )
                # fold this band's row sums and ship them while the next
                # band (or the epilogue) is still running
                b8 = band * 8
                nc.vector.tensor_reduce(
                    out=s_out_sb[:, b8 : b8 + 8],
                    in_=s_band[:, b8 : b8 + 8, :],
                    axis=mybir.AxisListType.X,
                    op=mybir.AluOpType.add,
                )
                nc.sync.dma_start(
                    out=s_dram[:, b8 : b8 + 8], in_=s_out_sb[:, b8 : b8 + 8]
                )

    nc.compile()
    return nc


def _get_nc():
    global _cached
    if _cached is None:
        _cached = _build()
    return _cached


def _prep(z_i: np.ndarray, z_j: np.ndarray):
    z = np.concatenate(
        [np.asarray(z_i, np.float32), np.asarray(z_j, np.float32)], axis=0
    )
    w = z * np.float32(math.sqrt(2.0))  # fold 1/T=2 into both operands
    wnorm = np.linalg.norm(w.astype(np.float64), axis=1)
    # per-kiloband exp offset from extreme-value estimate of the row max
    c_band = np.array(
        [
            ROWMAX_COEF * np.median(wnorm[b * KB : (b + 1) * KB])
            for b in range(16)
        ],
        dtype=np.float64,
    )
    in_maps = []
    for c in range(NCORES):
        wc = np.roll(w, -c * KB, axis=0)
        zT = np.ascontiguousarray(wc.T).astype(ml_dtypes.bfloat16)
        cneg = np.zeros((128, 4), dtype=np.float32)
        cneg[:, 0] = -c_band[c]
        cneg[:, 1] = -c_band[c + 8]
        cneg[:, 2] = BF16_ONE + EXP2_DELTA - c_band[c] * EXP2_A
        cneg[:, 3] = BF16_ONE + EXP2_DELTA - c_band[c + 8] * EXP2_A
        in_maps.append({"zT": zT, "c_neg": cneg})
    return w, c_band, in_maps


def _finish(w, c_band, results):
    s_abs = np.zeros(N, dtype=np.float64)
    # positives on the host: O(N*d), negligible next to the device's O(N^2*d)
    w64 = w.astype(np.float64)
    pos = (w64 * np.roll(w64, -TRAIN_NUM, axis=0)).sum(axis=1)
    for c in range(NCORES):
        r = results[c]
        s_dev = r["s_out"].astype(np.float64)      # [128, 16]
        col_dev = r["col_out"].astype(np.float64)  # [15, 2, 512]
        for band, kb in ((0, c), (1, c + 8)):
            scale = math.exp(c_band[kb])
            # row sums: s_dev[p, band*8+m] -> band row m*128+p
            rows = s_dev[:, band * 8 : band * 8 + 8].T.reshape(KB)  # [8*128]
            g0 = kb * KB
            s_abs[g0 : g0 + KB] += rows * scale
            # column sums from this band's chunks (D=0 chunk not exported)
            nchunks = 9 if band == 0 else 8
            for j in range(1, nchunks):
                for sl in range(2):
                    L = band * 8192 + j * CHUNK + sl * 512
                    vals = col_dev[band * 8 + (j - 1), sl, :] * scale
                    g = (c * KB + L) % N
                    s_abs[g : g + 512] += vals

    with np.errstate(divide="ignore", invalid="ignore"):
        lse = np.log(s_abs)
    bad = ~np.isfinite(lse)
    if bad.any():
        idx = np.nonzero(bad)[0]
        wb = w[idx].astype(np.float64)
        sim_b = wb @ w.astype(np.float64).T
        for ii, rr in enumerate(idx):
            sim_b[ii, rr] = -np.inf
        m_b = sim_b.max(axis=1)
        lse[idx] = np.log(np.exp(sim_b - m_b[:, None]).sum(axis=1)) + m_b
        pos_idx = np.where(idx < TRAIN_NUM, idx + TRAIN_NUM, idx - TRAIN_NUM)
        pos[idx] = np.einsum("ij,ij->i", wb, w[pos_idx].astype(np.float64))
    loss = (lse - pos).mean()
    return np.float32(loss)


def run(z_i, z_j, trace=False, **kw):
    from concourse.bass_utils import run_bass_kernel_spmd

    nc = _get_nc()
    w, c_band, in_maps = _prep(z_i, z_j)
    res = run_bass_kernel_spmd(
        nc, in_maps, core_ids=list(range(NCORES)), trace=trace, **kw
    )
    return _finish(w, c_band, res.results), res


def kernel(z_i, z_j):
    loss, _ = run(z_i, z_j, trace=False)
    return loss

